# revision 1
# baseline (speedup 1.0000x reference)
"""DeltaNet-style fast-weight kernel for Trainium2 (8 NeuronCores, data-parallel over batch).

Math (per batch element b):
  h = embed[seq]; x = h + MLP(h); h = LN(x)                      [L=512 tokens, H=64]
  keys k_t = h[t], t=0..510 ; kn_t = k_t/||k_t||
  M_t = M_{t-1}(I - kn_t kn_t^T) + k_t kn_t^T ; y = M_510 @ h[511]
  out = (y @ rp_w + rp_b) @ out_w + out_b

Key reduction: y = sum_t c_t k_t with a backward vector scan
  z_{510} = q;  c_t = kn_t . z_t;  z_{t-1} = z_t - c_t kn_t
(verified algebraically identical to the reference M-scan).
The kernel tracks zneg = -z so every scan op is a fused multiply-add:
  op1: u = (kn * -1) (x) zneg, accum -> c_t        (c_t = kn.z)
  op2: zneg' = (kn * c_t) + zneg                   (= -(z - c_t kn))
"""

import os
import sys

import numpy as np

for _p in ("/opt/trn_rl_repo", "/root/.axon_site/_ro/trn_rl_repo"):
    if os.path.isdir(_p) and _p not in sys.path:
        sys.path.insert(0, _p)

import concourse.bass as bass
import concourse.tile as tile
from concourse import bacc, mybir
from concourse.bass_utils import run_bass_kernel_spmd
from concourse.masks import make_identity

F32 = mybir.dt.float32
I32 = mybir.dt.int32
AF = mybir.ActivationFunctionType
OP = mybir.AluOpType

B, L, H, V = 256, 512, 64, 64
NCORES = 8
BS = B // NCORES          # 32 batches per core
LT = 16                   # l-values per tile
NT = L // LT              # 32 tiles
TPT = LT * BS             # 512 tokens per tile
LN_EPS = 1e-5


def _ap_bcast(dram_ap, nparts):
    """Partition-broadcast a DRAM AP across nparts partitions."""
    return bass.AP(
        tensor=dram_ap.tensor,
        offset=dram_ap.offset,
        ap=[[0, nparts], *dram_ap.ap],
    )


def build_program(ln_trivial: bool):
    # Bacc (not raw Bass): its finalize() runs move_matmul_waits_to_ldweights
    # + generate_event_semaphores, which split multi-semaphore waits to meet
    # the 1-wait-per-instruction TRN2 constraint.
    nc = bacc.Bacc(None, target_bir_lowering=False)

    seq_p = nc.declare_dram_parameter("seq", [BS, L], I32, isOutput=False)
    embed_p = nc.declare_dram_parameter("embed", [V, H], F32, isOutput=False)
    w1_p = nc.declare_dram_parameter("w1", [H, 2 * H], F32, isOutput=False)
    b1_p = nc.declare_dram_parameter("b1", [2 * H, 1], F32, isOutput=False)
    w2_p = nc.declare_dram_parameter("w2", [2 * H, H], F32, isOutput=False)
    b2_p = nc.declare_dram_parameter("b2", [H, 1], F32, isOutput=False)
    ln_g_p = nc.declare_dram_parameter("ln_g", [1, H], F32, isOutput=False)
    ln_b_p = nc.declare_dram_parameter("ln_b", [1, H], F32, isOutput=False)
    rp_w_p = nc.declare_dram_parameter("rp_w", [H, H], F32, isOutput=False)
    rp_b_p = nc.declare_dram_parameter("rp_b", [H, 1], F32, isOutput=False)
    out_w_p = nc.declare_dram_parameter("out_w", [H, V], F32, isOutput=False)
    out_b_p = nc.declare_dram_parameter("out_b", [V, 1], F32, isOutput=False)
    out_p = nc.declare_dram_parameter("out", [BS, V], F32, isOutput=True)

    # DRAM scratch: seq as f32, transposed to [L, BS] so the per-tile
    # partition-broadcast DMA reads contiguous 2KB runs.
    seqT_d = nc.dram_tensor("seqT_scratch", [L, BS], F32)

    from contextlib import ExitStack

    with tile.TileContext(nc) as tc, ExitStack() as ctx:
        consts = ctx.enter_context(tc.tile_pool(name="consts", bufs=1))
        big = ctx.enter_context(tc.tile_pool(name="big", bufs=1))
        work = ctx.enter_context(tc.tile_pool(name="work", bufs=2))
        ps_a = ctx.enter_context(tc.tile_pool(name="ps_a", bufs=2, space="PSUM"))
        ps_b = ctx.enter_context(tc.tile_pool(name="ps_b", bufs=2, space="PSUM"))
        ps_c = ctx.enter_context(tc.tile_pool(name="ps_c", bufs=1, space="PSUM"))
        ps_m = ctx.enter_context(tc.tile_pool(name="ps_m", bufs=1, space="PSUM"))

        # ---------------- constants / params ----------------
        ident = consts.tile([H, H], F32)
        make_identity(nc, ident)

        eps_sb = consts.tile([128, 1], F32)
        nc.vector.memset(eps_sb, LN_EPS)

        viota_i = consts.tile([V, 1], I32)
        nc.gpsimd.iota(viota_i, pattern=[[1, 1]], base=0, channel_multiplier=1)
        viota = consts.tile([V, 1], F32)
        nc.vector.tensor_copy(viota, viota_i)

        embed_sb = consts.tile([V, H], F32)
        w1_sb = consts.tile([H, 2 * H], F32)
        b1_sb = consts.tile([2 * H, 1], F32)
        w2_sb = consts.tile([2 * H, H], F32)
        b2_sb = consts.tile([H, 1], F32)
        rp_w_sb = consts.tile([H, H], F32)
        rp_b_sb = consts.tile([H, 1], F32)
        out_w_sb = consts.tile([H, V], F32)
        out_b_sb = consts.tile([V, 1], F32)
        for sb, p in (
            (embed_sb, embed_p), (w1_sb, w1_p), (b1_sb, b1_p), (w2_sb, w2_p),
            (b2_sb, b2_p), (rp_w_sb, rp_w_p), (rp_b_sb, rp_b_p),
            (out_w_sb, out_w_p), (out_b_sb, out_b_p),
        ):
            nc.sync.dma_start(out=sb, in_=p[:, :])

        if not ln_trivial:
            g_bc = consts.tile([128, H], F32)
            bta_bc = consts.tile([128, H], F32)
            nc.sync.dma_start(out=g_bc, in_=_ap_bcast(ln_g_p[0, :], 128))
            nc.sync.dma_start(out=bta_bc, in_=_ap_bcast(ln_b_p[0, :], 128))

        # seq -> f32, transpose to [L, BS], stash in DRAM
        seq_i = consts.tile([BS, L], I32)
        nc.sync.dma_start(out=seq_i, in_=seq_p[:, :])
        seq_f = consts.tile([BS, L], F32)
        nc.vector.tensor_copy(seq_f, seq_i)

        # PE transpose-mode matmuls lower to a single-wait-slot instruction.
        # This throwaway transpose depends only on the gpsimd-built identity,
        # advancing PE's observed Pool clock so later transposes need at most
        # one semaphore wait each.
        dummy_ps = ps_m.tile([BS, BS], F32, tag="psm_dummy")
        nc.tensor.matmul(dummy_ps, lhsT=ident[0:BS, 0:BS], rhs=ident[0:BS, 0:BS], start=True, stop=True)

        seqT_sb = consts.tile([128, 4, BS], F32)
        for k in range(4):
            pst = ps_m.tile([128, BS], F32, tag="psm")
            nc.tensor.matmul(pst, lhsT=seq_f[:, 128 * k:128 * (k + 1)], rhs=ident[0:BS, 0:BS], start=True, stop=True)
            nc.vector.tensor_copy(seqT_sb[:, k, :], pst)
        nc.sync.dma_start(
            out=seqT_d[:, :].rearrange("(k p) b -> p k b", p=128),
            in_=seqT_sb,
        )

        # embedT, then w1p = embed @ w1  (so a = onehot @ w1p directly)
        pse = ps_m.tile([H, H], F32, tag="psm")
        nc.tensor.matmul(pse, lhsT=embed_sb, rhs=ident, start=True, stop=True)
        embedT_sb = consts.tile([H, V], F32)
        nc.vector.tensor_copy(embedT_sb, pse)
        psw = ps_m.tile([V, 2 * H], F32, tag="psm")
        nc.tensor.matmul(psw, lhsT=embedT_sb, rhs=w1_sb, start=True, stop=True)
        w1p_sb = consts.tile([V, 2 * H], F32)
        nc.vector.tensor_copy(w1p_sb, psw)

        # ---------------- persistent big buffers ----------------
        # h (post-LN), phased token layout: partition 32*(l%4)+b, chunk l//4
        h_sb = big.tile([128, 128, H], F32)
        # kn, scan layout: [b, l, h] on partitions 0..31
        kn32 = big.tile([BS, L, H], F32)
        c_sb = big.tile([BS, L], F32)
        nc.vector.memset(c_sb, 0.0)

        # ---------------- per-tile pre-scan pipeline ----------------
        for i in range(NT):
            seqb = work.tile([V, TPT], F32)
            sl = seqT_d[LT * i:LT * (i + 1), :]
            nc.sync.dma_start(out=seqb, in_=_ap_bcast(sl, V))

            oh = work.tile([V, TPT], F32)
            nc.vector.tensor_scalar(
                out=oh, in0=seqb, scalar1=viota[:, 0:1], scalar2=None,
                op0=OP.is_equal,
            )

            psA = ps_a.tile([H, TPT], F32, tag="psA")     # hT, then xT
            psB = ps_b.tile([2 * H, TPT], F32, tag="psB")  # aT (pre-relu)
            nc.tensor.matmul(psB, lhsT=w1p_sb, rhs=oh, start=True, stop=True)

            rT = work.tile([2 * H, TPT], F32)
            nc.scalar.activation(rT, psB, AF.Relu, bias=b1_sb[:, 0:1])

            nc.tensor.matmul(psA, lhsT=embed_sb, rhs=oh, start=True, stop=False)
            nc.tensor.matmul(psA, lhsT=w2_sb, rhs=rT, start=False, stop=True)

            xT = work.tile([H, TPT], F32)
            nc.scalar.activation(xT, psA, AF.Identity, bias=b2_sb[:, 0:1])

            psC = ps_c.tile([128, 4, H], F32, tag="psC")
            for k in range(4):
                nc.tensor.matmul(psC[:, k, :], lhsT=xT[:, 128 * k:128 * (k + 1)], rhs=ident, start=True, stop=True)
            x_sb = work.tile([128, 4, H], F32)
            # on ACT: keeps the psC slot-release reader on the same engine as
            # the xT producer, so next tile's transposes carry a single wait
            nc.scalar.activation(x_sb, psC, AF.Copy)

            st = work.tile([128, 4, 6], F32)
            mv = work.tile([128, 4, 2], F32)
            for g in range(4):
                nc.vector.bn_stats(st[:, g, :], x_sb[:, g, :])
            for g in range(4):
                nc.vector.bn_aggr(mv[:, g, :], st[:, g, :])

            nrm = work.tile([128, 4, 1], F32)
            sstd = work.tile([128, 4, 1], F32)
            invn = work.tile([128, 4, 1], F32)
            rstd = work.tile([128, 4, 1], F32)
            var_ap = mv[:, :, 1:2]
            nc.scalar.activation(nrm, var_ap, AF.Sqrt, scale=float(H))
            nc.scalar.activation(sstd, var_ap, AF.Sqrt, bias=eps_sb[:, 0:1])
            nc.vector.reciprocal(invn, nrm)
            nc.vector.reciprocal(rstd, sstd)

            kn_t = work.tile([128, 4, H], F32)
            for g in range(4):
                mu = mv[:, g, 0:1]
                nc.vector.tensor_scalar(
                    out=h_sb[:, 4 * i + g, :], in0=x_sb[:, g, :],
                    scalar1=mu, scalar2=rstd[:, g, :],
                    op0=OP.subtract, op1=OP.mult,
                )
                if ln_trivial:
                    nc.vector.tensor_scalar(
                        out=kn_t[:, g, :], in0=x_sb[:, g, :],
                        scalar1=mu, scalar2=invn[:, g, :],
                        op0=OP.subtract, op1=OP.mult,
                    )
            if not ln_trivial:
                # h = h*g + beta, then kn = h/||h|| (general path)
                for g in range(4):
                    nc.vector.tensor_mul(h_sb[:, 4 * i + g, :], h_sb[:, 4 * i + g, :], g_bc)
                    nc.vector.tensor_add(h_sb[:, 4 * i + g, :], h_sb[:, 4 * i + g, :], bta_bc)
                ss = work.tile([128, 4, 1], F32)
                sn = work.tile([128, 4, 1], F32)
                rn = work.tile([128, 4, 1], F32)
                for g in range(4):
                    nc.vector.scalar_tensor_tensor(
                        out=kn_t[:, g, :], in0=h_sb[:, 4 * i + g, :], scalar=1.0,
                        in1=h_sb[:, 4 * i + g, :], op0=OP.mult, op1=OP.mult,
                        accum_out=ss[:, g, :],
                    )
                nc.scalar.activation(sn, ss, AF.Sqrt)
                nc.vector.tensor_scalar(sn, sn, 1e-12, None, op0=OP.max)
                nc.vector.reciprocal(rn, sn)
                for g in range(4):
                    nc.vector.tensor_scalar(
                        out=kn_t[:, g, :], in0=h_sb[:, 4 * i + g, :],
                        scalar1=rn[:, g, :], scalar2=None, op0=OP.mult,
                    )

            # bridge kn tile (phased) -> kn32 [b, l, h]
            for ph in range(4):
                dst = kn32[:, LT * i + ph:LT * (i + 1):4, :]
                nc.sync.dma_start(out=dst, in_=kn_t[32 * ph:32 * (ph + 1), :, :])

        # ---------------- backward scan ----------------
        zneg = big.tile([BS, H], F32)
        nc.sync.dma_start(out=zneg, in_=h_sb[96:128, 127, :])  # q = h[:, 511, :]
        nc.vector.tensor_scalar(zneg, zneg, -1.0, None, op0=OP.mult)

        u = big.tile([BS, H], F32)
        for l in range(L - 2, -1, -1):
            kn_ap = kn32[:, l, :]
            nc.vector.scalar_tensor_tensor(
                out=u, in0=kn_ap, scalar=-1.0, in1=zneg,
                op0=OP.mult, op1=OP.mult, accum_out=c_sb[:, l:l + 1],
            )
            nc.vector.scalar_tensor_tensor(
                out=zneg, in0=kn_ap, scalar=c_sb[:, l:l + 1], in1=zneg,
                op0=OP.mult, op1=OP.add,
            )

        # ---------------- y = sum_t c_t h_t (phased batch) ----------------
        c_rep = big.tile([128, 128], F32)
        for ph in range(4):
            nc.sync.dma_start(
                out=c_rep[32 * ph:32 * (ph + 1), :], in_=c_sb[:, ph::4],
            )

        y4 = big.tile([128, H], F32)
        nc.vector.memset(y4, 0.0)
        for ch in range(128):
            nc.vector.scalar_tensor_tensor(
                out=y4, in0=h_sb[:, ch, :], scalar=c_rep[:, ch:ch + 1],
                in1=y4, op0=OP.mult, op1=OP.add,
            )

        yt1 = big.tile([BS, H], F32)
        yt2 = big.tile([BS, H], F32)
        yt3 = big.tile([BS, H], F32)
        nc.sync.dma_start(out=yt1, in_=y4[32:64, :])
        nc.sync.dma_start(out=yt2, in_=y4[64:96, :])
        nc.sync.dma_start(out=yt3, in_=y4[96:128, :])
        y_sb = big.tile([BS, H], F32)
        nc.vector.tensor_add(y_sb, y4[0:BS, :], yt1)
        nc.vector.tensor_add(y_sb, y_sb, yt2)
        nc.vector.tensor_add(y_sb, y_sb, yt3)

        # ---------------- final projections ----------------
        psF = ps_m.tile([H, BS], F32, tag="psm")
        nc.tensor.matmul(psF, lhsT=y_sb, rhs=ident[0:BS, 0:BS], start=True, stop=True)
        yT = big.tile([H, BS], F32)
        nc.vector.tensor_copy(yT, psF)

        psG = ps_m.tile([H, BS], F32, tag="psm")
        nc.tensor.matmul(psG, lhsT=rp_w_sb, rhs=yT, start=True, stop=True)
        r1 = big.tile([H, BS], F32)
        nc.scalar.activation(r1, psG, AF.Identity, bias=rp_b_sb[:, 0:1])

        psH = ps_m.tile([V, BS], F32, tag="psm")
        nc.tensor.matmul(psH, lhsT=out_w_sb, rhs=r1, start=True, stop=True)
        r2 = big.tile([V, BS], F32)
        nc.scalar.activation(r2, psH, AF.Identity, bias=out_b_sb[:, 0:1])

        psI = ps_m.tile([BS, V], F32, tag="psm")
        nc.tensor.matmul(psI, lhsT=r2, rhs=ident, start=True, stop=True)
        o_sb = big.tile([BS, V], F32)
        nc.vector.tensor_copy(o_sb, psI)
        nc.sync.dma_start(out=out_p[:, :], in_=o_sb)

    nc.finalize()
    return nc


_CACHE = {}


def _run(inputs, trace=False, **kw):
    seq = np.asarray(inputs["seq"]).astype(np.int32)
    embed = np.asarray(inputs["embed"], np.float32)
    w1 = np.asarray(inputs["w1"], np.float32)
    b1 = np.asarray(inputs["b1"], np.float32).reshape(2 * H, 1)
    w2 = np.asarray(inputs["w2"], np.float32)
    b2 = np.asarray(inputs["b2"], np.float32).reshape(H, 1)
    ln_g = np.asarray(inputs["ln_g"], np.float32).reshape(1, H)
    ln_b = np.asarray(inputs["ln_b"], np.float32).reshape(1, H)
    rp_w = np.asarray(inputs["rp_w"], np.float32)
    rp_b = np.asarray(inputs["rp_b"], np.float32).reshape(H, 1)
    out_w = np.asarray(inputs["out_w"], np.float32)
    out_b = np.asarray(inputs["out_b"], np.float32).reshape(V, 1)

    ln_trivial = bool(np.all(ln_g == 1.0) and np.all(ln_b == 0.0))
    if ln_trivial not in _CACHE:
        _CACHE[ln_trivial] = build_program(ln_trivial)
    nc = _CACHE[ln_trivial]

    in_maps = []
    for c in range(NCORES):
        in_maps.append({
            "seq": seq[BS * c:BS * (c + 1)],
            "embed": embed, "w1": w1, "b1": b1, "w2": w2, "b2": b2,
            "ln_g": ln_g, "ln_b": ln_b,
            "rp_w": rp_w, "rp_b": rp_b, "out_w": out_w, "out_b": out_b,
        })
    br = run_bass_kernel_spmd(nc, in_maps, list(range(NCORES)), trace=trace, **kw)
    out = np.concatenate([r["out"] for r in br.results], axis=0)
    return out, br


def kernel(**inputs) -> np.ndarray:
    return _run(inputs)[0]



# revision 4
# speedup vs baseline: 1.6342x; 1.6342x over previous
"""DeltaNet-style fast-weight kernel for Trainium2 (8 NeuronCores, data-parallel over batch).

Math (per batch element b):
  h = embed[seq]; x = h + MLP(h); h = LN(x)                      [L=512 tokens, H=64]
  keys k_t = h[t], t=0..510 ; kn_t = k_t/||k_t||
  M_t = M_{t-1}(I - kn_t kn_t^T) + k_t kn_t^T ; y = M_510 @ h[511]
  out = (y @ rp_w + rp_b) @ out_w + out_b

Key reduction: y = sum_t c_t k_t with a backward vector scan
  z_{510} = q;  c_t = kn_t . z_t;  z_{t-1} = z_t - c_t kn_t
(verified algebraically identical to the reference M-scan).

Since ln_g=1/ln_b=0, h_t = s_t * kn_t with the per-token scalar
s_t = sqrt(H*var)/sqrt(var+eps), so only kn + s are materialized:
  y = sum_t (c_t s_t) kn_t ,  q = s_511 kn_511.

Tiles are produced in REVERSE token order so the serial backward scan
(the critical path) starts as soon as the last tokens' kn land and
overlaps the MLP pipeline for the remaining tiles.  y-accumulation and
the c*s fold run on the otherwise idle GpSimd engine.
"""

import os
import sys

import numpy as np

for _p in ("/opt/trn_rl_repo", "/root/.axon_site/_ro/trn_rl_repo"):
    if os.path.isdir(_p) and _p not in sys.path:
        sys.path.insert(0, _p)

import concourse.bass as bass
import concourse.tile as tile
from concourse import bacc, mybir
from concourse.bass_utils import run_bass_kernel_spmd
from concourse.masks import make_identity

F32 = mybir.dt.float32
BF16 = mybir.dt.bfloat16
I32 = mybir.dt.int32
AF = mybir.ActivationFunctionType
OP = mybir.AluOpType

B, L, H, V = 256, 512, 64, 64
NCORES = 8
BS = B // NCORES          # 32 batches per core
LT = 16                   # l-values per tile
NT = L // LT              # 32 tiles
TPT = LT * BS             # 512 tokens per tile
LN_EPS = 1e-5


def _ap_bcast(dram_ap, nparts):
    """Partition-broadcast a DRAM AP across nparts partitions."""
    return bass.AP(
        tensor=dram_ap.tensor,
        offset=dram_ap.offset,
        ap=[[0, nparts], *dram_ap.ap],
    )


def build_program(ln_trivial: bool):
    # Bacc (not raw Bass): its finalize() runs move_matmul_waits_to_ldweights
    # + generate_event_semaphores, which split multi-semaphore waits to meet
    # the 1-wait-per-instruction TRN2 constraint.
    nc = bacc.Bacc(None, target_bir_lowering=False)

    seq_p = nc.declare_dram_parameter("seq", [BS, L], I32, isOutput=False)
    embed_p = nc.declare_dram_parameter("embed", [V, H], F32, isOutput=False)
    w1_p = nc.declare_dram_parameter("w1", [H, 2 * H], F32, isOutput=False)
    b1_p = nc.declare_dram_parameter("b1", [2 * H, 1], F32, isOutput=False)
    w2_p = nc.declare_dram_parameter("w2", [2 * H, H], F32, isOutput=False)
    b2_p = nc.declare_dram_parameter("b2", [H, 1], F32, isOutput=False)
    ln_g_p = nc.declare_dram_parameter("ln_g", [1, H], F32, isOutput=False)
    ln_b_p = nc.declare_dram_parameter("ln_b", [1, H], F32, isOutput=False)
    rp_w_p = nc.declare_dram_parameter("rp_w", [H, H], F32, isOutput=False)
    rp_b_p = nc.declare_dram_parameter("rp_b", [H, 1], F32, isOutput=False)
    out_w_p = nc.declare_dram_parameter("out_w", [H, V], F32, isOutput=False)
    out_b_p = nc.declare_dram_parameter("out_b", [V, 1], F32, isOutput=False)
    out_p = nc.declare_dram_parameter("out", [BS, V], F32, isOutput=True)

    # DRAM scratch: seq as f32, transposed to [L, BS] so the per-tile
    # partition-broadcast DMA reads contiguous 2KB runs.
    seqT_d = nc.dram_tensor("seqT_scratch", [L, BS], F32)

    from contextlib import ExitStack

    with tile.TileContext(nc) as tc, ExitStack() as ctx:
        consts = ctx.enter_context(tc.tile_pool(name="consts", bufs=1))
        big = ctx.enter_context(tc.tile_pool(name="big", bufs=1))
        work = ctx.enter_context(tc.tile_pool(name="work", bufs=2))
        ps_a = ctx.enter_context(tc.tile_pool(name="ps_a", bufs=2, space="PSUM"))
        ps_b = ctx.enter_context(tc.tile_pool(name="ps_b", bufs=2, space="PSUM"))
        ps_c = ctx.enter_context(tc.tile_pool(name="ps_c", bufs=1, space="PSUM"))
        ps_m = ctx.enter_context(tc.tile_pool(name="ps_m", bufs=1, space="PSUM"))

        # ---------------- constants / params ----------------
        ident = consts.tile([H, H], F32)
        make_identity(nc, ident)

        eps_sb = consts.tile([128, 1], F32)
        nc.vector.memset(eps_sb, LN_EPS)

        viota_i = consts.tile([V, 1], I32)
        nc.gpsimd.iota(viota_i, pattern=[[1, 1]], base=0, channel_multiplier=1)
        viota = consts.tile([V, 1], F32)
        nc.vector.tensor_copy(viota, viota_i)

        embed_sb = consts.tile([V, H], F32)
        w1_sb = consts.tile([H, 2 * H], F32)
        b1_sb = consts.tile([2 * H, 1], F32)
        w2_sb = consts.tile([2 * H, H], F32)
        b2_sb = consts.tile([H, 1], F32)
        rp_w_sb = consts.tile([H, H], F32)
        rp_b_sb = consts.tile([H, 1], F32)
        out_w_sb = consts.tile([H, V], F32)
        out_b_sb = consts.tile([V, 1], F32)
        for sb, p in (
            (embed_sb, embed_p), (w1_sb, w1_p), (b1_sb, b1_p), (w2_sb, w2_p),
            (b2_sb, b2_p), (rp_w_sb, rp_w_p), (rp_b_sb, rp_b_p),
            (out_w_sb, out_w_p), (out_b_sb, out_b_p),
        ):
            nc.sync.dma_start(out=sb, in_=p[:, :])

        if not ln_trivial:
            g_bc = consts.tile([128, H], F32)
            bta_bc = consts.tile([128, H], F32)
            nc.sync.dma_start(out=g_bc, in_=_ap_bcast(ln_g_p[0, :], 128))
            nc.sync.dma_start(out=bta_bc, in_=_ap_bcast(ln_b_p[0, :], 128))

        # seq -> f32, transpose to [L, BS], stash in DRAM
        seq_i = consts.tile([BS, L], I32)
        nc.sync.dma_start(out=seq_i, in_=seq_p[:, :])
        seq_f = consts.tile([BS, L], F32)
        nc.vector.tensor_copy(seq_f, seq_i)

        # PE transpose-mode matmuls lower to a single-wait-slot instruction.
        # This throwaway transpose depends only on the gpsimd-built identity,
        # advancing PE's observed Pool clock so later transposes need at most
        # one semaphore wait each.
        dummy_ps = ps_m.tile([BS, BS], F32, tag="psm_dummy")
        nc.tensor.matmul(dummy_ps, lhsT=ident[0:BS, 0:BS], rhs=ident[0:BS, 0:BS], start=True, stop=True)

        seqT_sb = consts.tile([128, 4, BS], F32)
        for k in range(4):
            pst = ps_m.tile([128, BS], F32, tag="psm")
            nc.tensor.matmul(pst, lhsT=seq_f[:, 128 * k:128 * (k + 1)], rhs=ident[0:BS, 0:BS], start=True, stop=True)
            nc.vector.tensor_copy(seqT_sb[:, k, :], pst)
        nc.sync.dma_start(
            out=seqT_d[:, :].rearrange("(k p) b -> p k b", p=128),
            in_=seqT_sb,
        )

        # embedT, then w1p = embed @ w1 (fp32, once), then bf16 casts of the
        # per-token matmul weights (PE runs bf16 at 2x the fp32 rate).
        pse = ps_m.tile([H, H], F32, tag="psm")
        nc.tensor.matmul(pse, lhsT=embed_sb, rhs=ident, start=True, stop=True)
        embedT_sb = consts.tile([H, V], F32)
        nc.vector.tensor_copy(embedT_sb, pse)
        psw = ps_m.tile([V, 2 * H], F32, tag="psm")
        nc.tensor.matmul(psw, lhsT=embedT_sb, rhs=w1_sb, start=True, stop=True)
        w1p_sb = consts.tile([V, 2 * H], F32)
        nc.vector.tensor_copy(w1p_sb, psw)

        w1p_b = consts.tile([V, 2 * H], BF16)
        embed_b = consts.tile([V, H], BF16)
        w2_b = consts.tile([2 * H, H], BF16)
        nc.scalar.activation(w1p_b, w1p_sb, AF.Copy)
        nc.scalar.activation(embed_b, embed_sb, AF.Copy)
        nc.scalar.activation(w2_b, w2_sb, AF.Copy)

        # ---------------- persistent big buffers ----------------
        # kn in phased token layout (partition 32*(l%4)+b, chunk l//4) for
        # y-accumulation, and in scan layout [b, l, h] on partitions 0..31.
        kn_ph = big.tile([128, 128, H], F32)
        kn32 = big.tile([BS, L, H], F32)
        c_sb = big.tile([BS, L], F32)
        nc.vector.memset(c_sb, 0.0)
        s_all = big.tile([128, 128], F32)     # per-token scale s_t, phased
        c_rep = big.tile([128, 128], F32)     # c_t, phased
        w_all = big.tile([128, 128], F32)     # c_t * s_t, phased
        y4 = big.tile([128, H], F32)
        nc.vector.memset(y4, 0.0)
        zneg = big.tile([BS, H], F32)
        u = big.tile([BS, H], F32)

        # ---------------- per-tile pipeline, REVERSE order ----------------
        for i in range(NT - 1, -1, -1):
            seqb = work.tile([V, TPT], F32)
            sl = seqT_d[LT * i:LT * (i + 1), :]
            nc.sync.dma_start(out=seqb, in_=_ap_bcast(sl, V))

            oh = work.tile([V, TPT], BF16)
            nc.vector.tensor_scalar(
                out=oh, in0=seqb, scalar1=viota[:, 0:1], scalar2=None,
                op0=OP.is_equal,
            )

            psA = ps_a.tile([H, TPT], F32, tag="psA")     # hT, then xT
            psB = ps_b.tile([2 * H, TPT], F32, tag="psB")  # aT (pre-relu)
            nc.tensor.matmul(psB, lhsT=w1p_b, rhs=oh, start=True, stop=True)

            rT = work.tile([2 * H, TPT], BF16)
            nc.scalar.activation(rT, psB, AF.Relu, bias=b1_sb[:, 0:1])

            nc.tensor.matmul(psA, lhsT=embed_b, rhs=oh, start=True, stop=False)
            nc.tensor.matmul(psA, lhsT=w2_b, rhs=rT, start=False, stop=True)

            xT = work.tile([H, TPT], F32)
            nc.scalar.activation(xT, psA, AF.Identity, bias=b2_sb[:, 0:1])

            psC = ps_c.tile([128, 4, H], F32, tag="psC")
            for k in range(4):
                nc.tensor.matmul(psC[:, k, :], lhsT=xT[:, 128 * k:128 * (k + 1)], rhs=ident, start=True, stop=True)
            x_sb = work.tile([128, 4, H], F32)
            # on ACT: keeps the psC slot-release reader on the same engine as
            # the xT producer, so next tile's transposes carry a single wait
            nc.scalar.activation(x_sb, psC, AF.Copy)

            st = work.tile([128, 4, 6], F32)
            mv = work.tile([128, 4, 2], F32)
            for g in range(4):
                nc.vector.bn_stats(st[:, g, :], x_sb[:, g, :])
            for g in range(4):
                nc.vector.bn_aggr(mv[:, g, :], st[:, g, :])

            nrm = work.tile([128, 4, 1], F32)
            sstd = work.tile([128, 4, 1], F32)
            invn = work.tile([128, 4, 1], F32)
            rstd = work.tile([128, 4, 1], F32)
            var_ap = mv[:, :, 1:2]
            nc.scalar.activation(nrm, var_ap, AF.Sqrt, scale=float(H))
            nc.scalar.activation(sstd, var_ap, AF.Sqrt, bias=eps_sb[:, 0:1])
            nc.vector.reciprocal(invn, nrm)
            nc.vector.reciprocal(rstd, sstd)

            # s_t = ||x-mu|| * rstd = nrm * rstd   (h_t = s_t * kn_t)
            nc.vector.tensor_tensor(
                s_all[:, 4 * i:4 * (i + 1)], nrm[:, :, 0], rstd[:, :, 0], OP.mult,
            )

            kn_t = work.tile([128, 4, H], F32)
            for g in range(4):
                mu = mv[:, g, 0:1]
                nc.vector.tensor_scalar(
                    out=kn_t[:, g, :], in0=x_sb[:, g, :],
                    scalar1=mu, scalar2=invn[:, g, :],
                    op0=OP.subtract, op1=OP.mult,
                )
            if not ln_trivial:
                # general path: h = ((x-mu)*rstd)*g + beta, kn = h/||h||
                h_t = work.tile([128, 4, H], F32)
                for g in range(4):
                    mu = mv[:, g, 0:1]
                    nc.vector.tensor_scalar(
                        out=h_t[:, g, :], in0=x_sb[:, g, :],
                        scalar1=mu, scalar2=rstd[:, g, :],
                        op0=OP.subtract, op1=OP.mult,
                    )
                    nc.vector.tensor_mul(h_t[:, g, :], h_t[:, g, :], g_bc)
                    nc.vector.tensor_add(h_t[:, g, :], h_t[:, g, :], bta_bc)
                ss = work.tile([128, 4, 1], F32)
                sn = work.tile([128, 4, 1], F32)
                rn = work.tile([128, 4, 1], F32)
                for g in range(4):
                    nc.vector.scalar_tensor_tensor(
                        out=kn_t[:, g, :], in0=h_t[:, g, :], scalar=1.0,
                        in1=h_t[:, g, :], op0=OP.mult, op1=OP.mult,
                        accum_out=ss[:, g, :],
                    )
                nc.scalar.activation(sn, ss, AF.Sqrt)
                nc.vector.tensor_scalar(sn, sn, 1e-12, None, op0=OP.max)
                nc.vector.reciprocal(rn, sn)
                for g in range(4):
                    nc.vector.tensor_scalar(
                        out=kn_t[:, g, :], in0=h_t[:, g, :],
                        scalar1=rn[:, g, :], scalar2=None, op0=OP.mult,
                    )
                # s_t = ||h|| (y-accum uses h = s*kn)
                nc.scalar.activation(s_all[:, 4 * i:4 * (i + 1)], sn[:, :, 0], AF.Copy)

            # kn into phased buffer (for y-accum) and scan layout [b, l, h]
            nc.vector.tensor_copy(kn_ph[:, 4 * i:4 * (i + 1), :], kn_t)
            for ph in range(4):
                dst = kn32[:, LT * i + ph:LT * (i + 1):4, :]
                nc.sync.dma_start(out=dst, in_=kn_t[32 * ph:32 * (ph + 1), :, :])

            # ------------- backward scan steps for this tile -------------
            if i == NT - 1:
                # q = h[:,511,:] = s_511 * kn_511 ;  zneg = -q
                nc.vector.tensor_scalar(
                    out=zneg, in0=kn32[:, L - 1, :],
                    scalar1=s_all[96:128, 127:128], scalar2=-1.0,
                    op0=OP.mult, op1=OP.mult,
                )
                ls = range(L - 2, LT * i - 1, -1)
            else:
                ls = range(LT * (i + 1) - 1, LT * i - 1, -1)
            for l in ls:
                kn_ap = kn32[:, l, :]
                nc.vector.scalar_tensor_tensor(
                    out=u, in0=kn_ap, scalar=-1.0, in1=zneg,
                    op0=OP.mult, op1=OP.mult, accum_out=c_sb[:, l:l + 1],
                )
                nc.vector.scalar_tensor_tensor(
                    out=zneg, in0=kn_ap, scalar=c_sb[:, l:l + 1], in1=zneg,
                    op0=OP.mult, op1=OP.add,
                )

            # ------------- y accumulation for this tile (GpSimd) -------------
            for ph in range(4):
                nc.sync.dma_start(
                    out=c_rep[32 * ph:32 * (ph + 1), 4 * i:4 * (i + 1)],
                    in_=c_sb[:, LT * i + ph:LT * (i + 1):4],
                )
            nc.vector.tensor_tensor(
                w_all[:, 4 * i:4 * (i + 1)],
                c_rep[:, 4 * i:4 * (i + 1)],
                s_all[:, 4 * i:4 * (i + 1)], OP.mult,
            )
            for ch in range(4 * i + 3, 4 * i - 1, -1):
                nc.vector.scalar_tensor_tensor(
                    out=y4, in0=kn_ph[:, ch, :], scalar=w_all[:, ch:ch + 1],
                    in1=y4, op0=OP.mult, op1=OP.add,
                )

        # ---------------- reduce phases + final projections ----------------
        yt1 = big.tile([BS, H], F32)
        yt2 = big.tile([BS, H], F32)
        yt3 = big.tile([BS, H], F32)
        nc.sync.dma_start(out=yt1, in_=y4[32:64, :])
        nc.sync.dma_start(out=yt2, in_=y4[64:96, :])
        nc.sync.dma_start(out=yt3, in_=y4[96:128, :])
        y_sb = big.tile([BS, H], F32)
        nc.vector.tensor_add(y_sb, y4[0:BS, :], yt1)
        nc.vector.tensor_add(y_sb, y_sb, yt2)
        nc.vector.tensor_add(y_sb, y_sb, yt3)

        psF = ps_m.tile([H, BS], F32, tag="psm")
        nc.tensor.matmul(psF, lhsT=y_sb, rhs=ident[0:BS, 0:BS], start=True, stop=True)
        yT = big.tile([H, BS], F32)
        nc.vector.tensor_copy(yT, psF)

        psG = ps_m.tile([H, BS], F32, tag="psm")
        nc.tensor.matmul(psG, lhsT=rp_w_sb, rhs=yT, start=True, stop=True)
        r1 = big.tile([H, BS], F32)
        nc.scalar.activation(r1, psG, AF.Identity, bias=rp_b_sb[:, 0:1])

        psH = ps_m.tile([V, BS], F32, tag="psm")
        nc.tensor.matmul(psH, lhsT=out_w_sb, rhs=r1, start=True, stop=True)
        r2 = big.tile([V, BS], F32)
        nc.scalar.activation(r2, psH, AF.Identity, bias=out_b_sb[:, 0:1])

        psI = ps_m.tile([BS, V], F32, tag="psm")
        nc.tensor.matmul(psI, lhsT=r2, rhs=ident, start=True, stop=True)
        o_sb = big.tile([BS, V], F32)
        nc.vector.tensor_copy(o_sb, psI)
        nc.sync.dma_start(out=out_p[:, :], in_=o_sb)

    nc.finalize()
    return nc


_CACHE = {}


def _run(inputs, trace=False, **kw):
    seq = np.asarray(inputs["seq"]).astype(np.int32)
    embed = np.asarray(inputs["embed"], np.float32)
    w1 = np.asarray(inputs["w1"], np.float32)
    b1 = np.asarray(inputs["b1"], np.float32).reshape(2 * H, 1)
    w2 = np.asarray(inputs["w2"], np.float32)
    b2 = np.asarray(inputs["b2"], np.float32).reshape(H, 1)
    ln_g = np.asarray(inputs["ln_g"], np.float32).reshape(1, H)
    ln_b = np.asarray(inputs["ln_b"], np.float32).reshape(1, H)
    rp_w = np.asarray(inputs["rp_w"], np.float32)
    rp_b = np.asarray(inputs["rp_b"], np.float32).reshape(H, 1)
    out_w = np.asarray(inputs["out_w"], np.float32)
    out_b = np.asarray(inputs["out_b"], np.float32).reshape(V, 1)

    ln_trivial = bool(np.all(ln_g == 1.0) and np.all(ln_b == 0.0))
    if ln_trivial not in _CACHE:
        _CACHE[ln_trivial] = build_program(ln_trivial)
    nc = _CACHE[ln_trivial]

    in_maps = []
    for c in range(NCORES):
        in_maps.append({
            "seq": seq[BS * c:BS * (c + 1)],
            "embed": embed, "w1": w1, "b1": b1, "w2": w2, "b2": b2,
            "ln_g": ln_g, "ln_b": ln_b,
            "rp_w": rp_w, "rp_b": rp_b, "out_w": out_w, "out_b": out_b,
        })
    br = run_bass_kernel_spmd(nc, in_maps, list(range(NCORES)), trace=trace, **kw)
    out = np.concatenate([r["out"] for r in br.results], axis=0)
    return out, br


def kernel(**inputs) -> np.ndarray:
    return _run(inputs)[0]
